# revision 12
# baseline (speedup 1.0000x reference)
"""DeepseekV2 MLA decoder-layer attention on 8 Trainium2 NeuronCores.

Distribution (tensor-parallel over heads):
  - A-projection is sequence-sharded (256 tokens/core). The rmsnorms are NOT
    applied by the producer: raw latents + the per-token 1/rms rows are
    AllGathered, and the scaling is folded into the B-projection consumers
    (rinv commutes with the linear projections). This lets the gathers start
    the moment the A-proj matmuls finish, with no DVE normalization pass on
    the critical path.
  - Three pipelined collectives: AG1 = kv latents + roped k_pe + rinv_kv
    (issued ~1/3 into the A-proj), AG2a = q latents chunks 0-5, AG2b =
    chunks 6-11 + rinv_q. kv/v projections overlap the q gathers.
  - B-projections, flash attention and o_proj are head-sharded (2 heads/core,
    both heads processed together):
      * q rope-projection pairs are packed into one PE pass via column tiling
        (h0 -> psum rows 0:64, h1 -> rows 64:128).
      * flash rope-score pairs are packed via row tiling (kpe duplicated on
        partitions 0:64 / 64:128), softmax-denominator (z) pairs via column
        tiling, and the two heads' exp() runs as one 2-bank [128,1024] ACT op.
      * diagonal causal chunks only compute live score columns.
  - o_proj partials (fp16) are summed on host.
"""
import numpy as np

import concourse.bass as bass
import concourse.mybir as mybir
import concourse.tile as tile
from concourse import bacc
from concourse.bass_utils import run_bass_kernel_spmd

HIDDEN = 2048
H = 16
NOPE = 128
ROPE = 64
VDIM = 128
QLR = 1536
KVLR = 512
QK = NOPE + ROPE            # 192
THETA = 10000.0
EPS = 1e-6
SEQ = 2048

N_CORES = 8
HPC = H // N_CORES          # 2 heads per core
SSH = SEQ // N_CORES        # 256-token shard
P = 128

F32 = mybir.dt.float32
F32R = mybir.dt.float32r
F16 = mybir.dt.float16
EXP_BIAS = -4.0             # exp(x*scale + EXP_BIAS): cancels in softmax ratio

SCALE = float(QK) ** -0.5
NEG = -60000.0              # fits fp16; exp(scale*(s+NEG)+bias) == 0

N_KC = HIDDEN // P          # 16
N_QAC = QLR // P            # 12
N_KVC = KVLR // P           # 4
N_SB = SEQ // 512           # 4 query blocks
N_SC = SEQ // P             # 16


def build_program():
    nc = bacc.Bacc("TRN2", target_bir_lowering=False, debug=False,
                   num_devices=N_CORES)

    h1 = nc.dram_tensor("h1", [P, N_KC, SSH], F16, kind="ExternalInput")
    w1 = nc.dram_tensor("w1", [17, P, HIDDEN], F16, kind="ExternalInput")
    wq = nc.dram_tensor("wq", [P, N_QAC, HPC * QK], F16, kind="ExternalInput")
    wkv = nc.dram_tensor("wkv", [P, N_KVC, HPC * (NOPE + VDIM)], F16, kind="ExternalInput")
    wo = nc.dram_tensor("wo", [P, HPC, HIDDEN], F16, kind="ExternalInput")
    csq2 = nc.dram_tensor("csq2", [P, SEQ], F16, kind="ExternalInput")
    ssq2 = nc.dram_tensor("ssq2", [P, SEQ], F16, kind="ExternalInput")
    clc = nc.dram_tensor("clc", [ROPE, SSH], F16, kind="ExternalInput")
    cls = nc.dram_tensor("cls", [ROPE, SSH], F16, kind="ExternalInput")
    psw1 = nc.dram_tensor("psw1", [ROPE, ROPE], F16, kind="ExternalInput")
    psw2 = nc.dram_tensor("psw2", [P, P], F16, kind="ExternalInput")
    onesc_d = nc.dram_tensor("onesc", [P, 1], F16, kind="ExternalInput")
    onesr_d = nc.dram_tensor("onesr", [1, P], F16, kind="ExternalInput")
    onesrr_d = nc.dram_tensor("onesrr", [33, P], F32R, kind="ExternalInput")
    yout = nc.dram_tensor("y", [SEQ, HIDDEN], F16, kind="ExternalOutput")

    with tile.TileContext(nc) as tc:
        _emit(nc, tc, h1, w1, wq, wkv, wo, csq2, ssq2, clc, cls, psw1, psw2,
              onesc_d, onesr_d, onesrr_d, yout)
    nc.compile()
    return nc


def _emit(nc, tc, h1, w1, wq, wkv, wo, csq2_d, ssq2_d, clc_d, cls_d, psw1_d,
          psw2_d, onesc_d, onesr_d, onesrr_d, yout):
    Exp = mybir.ActivationFunctionType.Exp
    Sqrt = mybir.ActivationFunctionType.Sqrt
    rg = [list(range(N_CORES))]

    with tc.tile_pool(name="const", bufs=1) as const, \
         tc.tile_pool(name="att", bufs=1) as att, \
         tc.tile_pool(name="work", bufs=2) as work, \
         tc.tile_pool(name="lstr", bufs=4) as lstr, \
         tc.tile_pool(name="epool", bufs=4) as epool, \
         tc.tile_pool(name="dram", bufs=1, space="DRAM") as dram:

        # ---- first-matmul operands lead the DMA queues ----
        psw2t = const.tile([P, P], F16)
        nc.sync.dma_start(psw2t[:], psw2_d[:])
        hloc = att.tile([P, N_KC, SSH], F16)
        nc.sync.dma_start(hloc[:, 0:4, :], h1[:, 0:4, :])
        wt0 = att.tile([P, HIDDEN], F16, name="w1t", tag="w1t", bufs=3)
        for pc in range(4):
            sl = slice(pc * 512, (pc + 1) * 512)
            nc.sync.dma_start(wt0[:, sl], w1[12][:, sl])
        nc.sync.dma_start(hloc[:, 4:10, :], h1[:, 4:10, :])
        nc.sync.dma_start(hloc[:, 10:16, :], h1[:, 10:16, :])

        # ---- small constants ----
        onesc_f = const.tile([P, 1], F16)
        nc.sync.dma_start(onesc_f[:], onesc_d[:])
        onesr_f = const.tile([1, P], F16)
        nc.sync.dma_start(onesr_f[:], onesr_d[:])
        onesr33 = const.tile([33, P], F32R)
        nc.sync.dma_start(onesr33[:], onesrr_d[:])
        psw1t = const.tile([ROPE, ROPE], F16)
        nc.sync.dma_start(psw1t[:], psw1_d[:])
        cl = const.tile([ROPE, 2, SSH], F16)
        nc.sync.dma_start(cl[:, 0, :], clc_d[:])
        nc.sync.dma_start(cl[:, 1, :], cls_d[:])
        eps1 = const.tile([1, 1], F32)
        nc.vector.memset(eps1[:], EPS)
        negc = const.tile([P, 1], F32)
        nc.vector.memset(negc[:], EXP_BIAS)
        masks = const.tile([P, 4, 2, 512], F16)
        for j in range(4):
            for hh in range(2):
                nc.vector.memset(masks[:, j, hh, :], 0.0)
                nc.gpsimd.affine_select(
                    out=masks[:, j, hh, :], in_=masks[:, j, hh, :],
                    compare_op=mybir.AluOpType.is_ge, fill=NEG,
                    base=-128 * j, pattern=[[1, 512]], channel_multiplier=-1,
                )

        # ---- AllGather payload staging (DRAM) ----
        ag_in1 = dram.tile([P, 5, SSH], F16)        # kv 0..3 | kpe(0:64)+rinv_kv(64)
        ag_in2a = dram.tile([P, 6, SSH], F16)       # q chunks 0..5
        ag_in2b = dram.tile([P, 7, SSH], F16)       # q chunks 6..11 | rinv_q

        # =============== phase 1: A-proj + rope(k_pe) + rinv ===============
        with tc.tile_pool(name="ps1", bufs=1, space="PSUM") as ps1:
            # dense warm-up burst: flips the PE HAM clock gate to 8/8 during
            # the initial DMA ramp (PE would otherwise idle here)
            for _ in range(28):
                wu = ps1.tile([P, P], F32, tag="qacc", name="wu", bufs=3)
                nc.tensor.matmul(wu[:], psw2t[:], psw2t[:], start=True, stop=True)
            ss_q = ps1.tile([1, SSH], F32, tag="ssq", name="ss_q")
            ss_kv = ps1.tile([1, SSH], F32, tag="sskv", name="ss_kv")

            m_order = [12, 13, 14, 15, 16] + list(range(N_QAC))
            for mi, m in enumerate(m_order):
                acc = ps1.tile([P, SSH], F32, tag="qacc", name="a_acc", bufs=3)
                if mi == 0:
                    wt = wt0
                else:
                    wt = att.tile([P, HIDDEN], F16, name="w1t", tag="w1t", bufs=3)
                    nc.sync.dma_start(wt[:], w1[m])
                for k in range(N_KC):
                    nc.tensor.matmul(acc[:], wt[:, k * P:(k + 1) * P], hloc[:, k, :],
                                     start=(k == 0), stop=(k == N_KC - 1))
                if m < 16:
                    sq = work.tile([P, SSH], F16, name="sq")
                    with nc.allow_low_precision(reason="squares of O(1) values"):
                        nc.scalar.square(sq[:], acc[:])
                    tgt = ss_q if m < N_QAC else ss_kv
                    nc.tensor.matmul(tgt[:], onesc_f[:], sq[:],
                                     start=(m == 0) or (m == 12),
                                     stop=(m == N_QAC - 1) or (m == 15))
                # evict raw latents to fp16 + stream to the gather payloads
                if m < N_QAC:
                    lt = work.tile([P, SSH], F16, name="lt", bufs=4)
                    with nc.allow_low_precision(reason="fp16 gather payload"):
                        nc.vector.tensor_copy(lt[:], acc[:])
                    if m < 6:
                        nc.sync.dma_start(ag_in2a[:, m, :], lt[:])
                    else:
                        nc.sync.dma_start(ag_in2b[:, m - 6, :], lt[:])
                elif m < 16:
                    lt = work.tile([P, SSH], F16, name="lt", bufs=4)
                    with nc.allow_low_precision(reason="fp16 gather payload"):
                        nc.vector.tensor_copy(lt[:], acc[:])
                    nc.sync.dma_start(ag_in1[:, m - 12, :], lt[:])
                else:
                    # k_pe: rope locally (linear ops; not rms-normed)
                    kpl = work.tile([ROPE, SSH], F16, name="kpl", bufs=1)
                    with nc.allow_low_precision(reason="fp16 rope operand"):
                        nc.vector.tensor_copy(kpl[:], acc[:ROPE, :])
                    swp = ps1.tile([ROPE, SSH], F32, tag="qacc", name="swp", bufs=3)
                    nc.tensor.matmul(swp[:], psw1t[:], kpl[:], start=True, stop=True)
                    t1 = work.tile([ROPE, SSH], F16, name="t1", bufs=1)
                    sws = work.tile([ROPE, SSH], F16, name="sws", bufs=1)
                    kpo = work.tile([ROPE, SSH], F16, name="kpo", bufs=1)
                    with nc.allow_low_precision(reason="fp16 rope math"):
                        nc.vector.tensor_mul(t1[:], kpl[:], cl[:, 0, :])
                        nc.vector.tensor_mul(sws[:], swp[:], cl[:, 1, :])
                        nc.vector.tensor_add(kpo[:], t1[:], sws[:])
                    nc.sync.dma_start(ag_in1[:ROPE, 4, :], kpo[:])

                if m == 15:
                    # rinv_kv = 1/sqrt(mean(kv_a^2) + eps)
                    rt = work.tile([1, SSH], F32, name="rtkv", bufs=1)
                    nc.scalar.activation(rt[:], ss_kv[:], Sqrt, bias=eps1[:],
                                         scale=1.0 / KVLR)
                    rikv = work.tile([1, SSH], F16, name="rikv", bufs=1)
                    with nc.allow_low_precision(reason="fp16 rinv payload"):
                        nc.vector.reciprocal(rikv[:], rt[:])
                    nc.sync.dma_start(ag_in1[ROPE:ROPE + 1, 4, :], rikv[:])
                if m == 16:
                    ag_out1 = dram.tile([N_CORES, P, 5 * SSH], F16, addr_space="Shared")
                    nc.gpsimd.collective_compute(
                        "AllGather", mybir.AluOpType.bypass, replica_groups=rg,
                        ins=[ag_in1[:].rearrange("p m s -> p (m s)").opt()],
                        outs=[ag_out1.opt()],
                    )
                if m == 5:
                    ag_out2a = dram.tile([N_CORES, P, 6 * SSH], F16, addr_space="Shared")
                    nc.gpsimd.collective_compute(
                        "AllGather", mybir.AluOpType.bypass, replica_groups=rg,
                        ins=[ag_in2a[:].rearrange("p m s -> p (m s)").opt()],
                        outs=[ag_out2a.opt()],
                    )
                if m == N_QAC - 1:
                    rtq = work.tile([1, SSH], F32, name="rtq", bufs=1)
                    nc.scalar.activation(rtq[:], ss_q[:], Sqrt, bias=eps1[:],
                                         scale=1.0 / QLR)
                    riq = work.tile([1, SSH], F16, name="riq", bufs=1)
                    with nc.allow_low_precision(reason="fp16 rinv payload"):
                        nc.vector.reciprocal(riq[:], rtq[:])
                    nc.sync.dma_start(ag_in2b[0:1, 6, :], riq[:])
                    ag_out2b = dram.tile([N_CORES, P, 7 * SSH], F16, addr_space="Shared")
                    nc.gpsimd.collective_compute(
                        "AllGather", mybir.AluOpType.bypass, replica_groups=rg,
                        ins=[ag_in2b[:].rearrange("p m s -> p (m s)").opt()],
                        outs=[ag_out2b.opt()],
                    )

            agv1 = ag_out1[:].rearrange("c p (m s) -> c p m s", m=5)
            agv2a = ag_out2a[:].rearrange("c p (m s) -> c p m s", m=6)
            agv2b = ag_out2b[:].rearrange("c p (m s) -> c p m s", m=7)

            # ---- B-proj weights + rope tables (DMA overlaps the gathers) ----
            wkvs = att.tile([P, N_KVC, HPC * (NOPE + VDIM)], F16)
            nc.sync.dma_start(wkvs[:], wkv[:])
            wqs = att.tile([P, N_QAC, HPC * QK], F16)
            nc.sync.dma_start(wqs[:], wq[:])
            wos = att.tile([P, HPC, HIDDEN], F16)
            nc.sync.dma_start(wos[:], wo[:])
            csq2 = att.tile([P, SEQ], F16)
            nc.sync.dma_start(csq2[:], csq2_d[:])
            ssq2 = att.tile([P, SEQ], F16)
            nc.sync.dma_start(ssq2[:], ssq2_d[:])

            # ---- gathered rinv rows + kpe (both head-copies) ----
            kpe2 = att.tile([P, SEQ], F16)
            nc.sync.dma_start(kpe2[:ROPE, :].rearrange("p (c s) -> p c s", c=N_CORES),
                              agv1[:, :ROPE, 4, :].rearrange("c p s -> p c s"))
            nc.sync.dma_start(kpe2[ROPE:, :].rearrange("p (c s) -> p c s", c=N_CORES),
                              agv1[:, :ROPE, 4, :].rearrange("c p s -> p c s"))
            rikv_row = att.tile([1, SEQ], F16)
            nc.sync.dma_start(rikv_row[:].rearrange("p (c s) -> p c s", c=N_CORES),
                              agv1[:, ROPE:ROPE + 1, 4, :].rearrange("c p s -> p c s"))

            kn = [att.tile([P, SEQ], F16, name=f"kn{h}") for h in range(HPC)]
            qn = [att.tile([P, SEQ], F16, name=f"qn{h}") for h in range(HPC)]
            qpb2 = att.tile([P, SEQ], F16)
            vv = att.tile([P, N_SC, HPC * VDIM], F16)
            ao = [att.tile([P, SEQ], F16, name=f"ao{h}") for h in range(HPC)]

            # keep the PE clock gate warm through the AG1 rendezvous wait
            for _ in range(12):
                wu = ps1.tile([P, 512], F32, tag="qacc", name="wu2", bufs=3)
                nc.tensor.matmul(wu[:], psw2t[:], csq2[:, 0:512], start=True, stop=True)

            # =============== phase 2: kv B-proj (overlaps AG2a/b) ==========
            for nb in range(N_SB):
                sblk = slice(nb * 512, (nb + 1) * 512)
                c0 = nb * 2
                bck = ps1.tile([P, 512], F32, tag="qacc", name="bck", bufs=3)
                nc.tensor.matmul(bck[:], onesr_f[:], rikv_row[0:1, sblk],
                                 start=True, stop=True)
                bcks = work.tile([P, 512], F16, name="bcks")
                with nc.allow_low_precision(reason="fp16 rinv bcast"):
                    nc.vector.tensor_copy(bcks[:], bck[:])
                kva = []
                for k in range(N_KVC):
                    kt = lstr.tile([P, 2, SSH], F16, name="kva", bufs=8)
                    nc.sync.dma_start(
                        kt[:], agv1[c0:c0 + 2, :, k, :].rearrange("c p s -> p c s"))
                    kv_ = kt[:].rearrange("p c s -> p (c s)")
                    with nc.allow_low_precision(reason="rmsnorm fold, fp16"):
                        nc.vector.tensor_mul(kv_, kv_, bcks[:])
                    kva.append(kv_)
                for h in range(HPC):
                    acc = ps1.tile([P, 512], F32, tag="qacc", name="kn_acc", bufs=3)
                    for k in range(N_KVC):
                        nc.tensor.matmul(acc[:], wkvs[:, k, h * NOPE:(h + 1) * NOPE],
                                         kva[k], start=(k == 0), stop=(k == N_KVC - 1))
                    with nc.allow_low_precision(reason="fp16 flash operand"):
                        nc.vector.tensor_copy(kn[h][:, sblk], acc[:])
                for tsub in range(4):
                    t_idx = nb * 4 + tsub
                    acc = ps1.tile([P, HPC * VDIM], F32, tag="qacc", name="v_acc", bufs=3)
                    for k in range(N_KVC):
                        nc.tensor.matmul(
                            acc[:], kva[k][:, tsub * P:(tsub + 1) * P],
                            wkvs[:, k, HPC * NOPE:], start=(k == 0), stop=(k == N_KVC - 1))
                    with nc.allow_low_precision(reason="fp16 flash operand"):
                        nc.vector.tensor_copy(vv[:, t_idx, :], acc[:])

            # =============== phase 3: q B-proj (rope pair col-packed) ======
            # (emitted after the kv loads so its AG2b wait can't head-of-line
            # block the AG1-gated kva DMAs)
            riq_row = att.tile([1, SEQ], F16)
            nc.sync.dma_start(riq_row[:].rearrange("p (c s) -> p c s", c=N_CORES),
                              agv2b[:, 0:1, 6, :].rearrange("c p s -> p c s"))
            for nb in range(N_SB):
                sblk = slice(nb * 512, (nb + 1) * 512)
                c0 = nb * 2
                an0 = ps1.tile([P, 512], F32, tag="qn0", name="an0")
                an1 = ps1.tile([P, 512], F32, tag="qn1", name="an1")
                ap2 = ps1.tile([P, 512], F32, tag="qp2", name="ap2")
                for k in range(N_QAC):
                    qa3 = lstr.tile([P, 2, SSH], F16, name="qa", bufs=8)
                    if k < 6:
                        src = agv2a[c0:c0 + 2, :, k, :]
                    else:
                        src = agv2b[c0:c0 + 2, :, k - 6, :]
                    nc.sync.dma_start(qa3[:], src.rearrange("c p s -> p c s"))
                    qa = qa3[:].rearrange("p c s -> p (c s)")
                    st, sp = (k == 0), (k == N_QAC - 1)
                    col = 0
                    nc.tensor.matmul(an0[:], wqs[:, k, 0:NOPE], qa, start=st, stop=sp)
                    nc.tensor.matmul(an1[:], wqs[:, k, NOPE:2 * NOPE], qa, start=st, stop=sp)
                    nc.tensor.matmul(ap2[:ROPE, :], wqs[:, k, 2 * NOPE:2 * NOPE + ROPE],
                                     qa, start=st, stop=sp)
                    nc.tensor.matmul(ap2[ROPE:, :], wqs[:, k, 2 * NOPE + ROPE:],
                                     qa, start=st, stop=sp)
                # evictions: fold rinv_q; rope on the packed [h0;h1] rope rows
                bcq = ps1.tile([P, 512], F32, tag="qacc", name="bcq", bufs=3)
                nc.tensor.matmul(bcq[:], onesr_f[:], riq_row[0:1, sblk],
                                 start=True, stop=True)
                bcqs = work.tile([P, 512], F16, name="bcqs")
                with nc.allow_low_precision(reason="fp16 rinv bcast"):
                    nc.vector.tensor_copy(bcqs[:], bcq[:])
                with nc.allow_low_precision(reason="rmsnorm fold, fp16"):
                    nc.vector.tensor_mul(qn[0][:, sblk], an0[:], bcqs[:])
                    nc.vector.tensor_mul(qn[1][:, sblk], an1[:], bcqs[:])
                qp2s = work.tile([P, 512], F16, name="qp2s")
                with nc.allow_low_precision(reason="rmsnorm fold, fp16"):
                    nc.vector.tensor_mul(qp2s[:], ap2[:], bcqs[:])
                swp2 = ps1.tile([P, 512], F32, tag="qacc", name="swp2", bufs=3)
                nc.tensor.matmul(swp2[:], psw2t[:], qp2s[:], start=True, stop=True)
                sw2 = work.tile([P, 512], F16, name="sw2")
                t2 = work.tile([P, 512], F16, name="t2")
                with nc.allow_low_precision(reason="fp16 rope math"):
                    nc.vector.tensor_mul(sw2[:], swp2[:], ssq2[:, sblk])
                    nc.vector.tensor_mul(t2[:], qp2s[:], csq2[:, sblk])
                    nc.vector.tensor_add(qpb2[:, sblk], t2[:], sw2[:])

        # =============== phase 4: flash attention + o_proj =================
        with tc.tile_pool(name="ps2", bufs=1, space="PSUM") as ps2:
            # init the two rotating score psum slots (stale-NaN guard for
            # the triangular-trimmed diagonal chunks)
            for _ in range(2):
                si = ps2.tile([P, 2, 512], F32, tag="s2", name="s_init", bufs=2)
                nc.vector.memset(si[:], 0.0)

            chunks = [(b, t) for b in range(N_SB) for t in range(4 * (b + 1))]
            exq = []

            def emit_scores(b, t):
                sblk0 = b * 512
                j = t - 4 * b
                f0 = 128 * j if j >= 0 else 0
                tsl = slice(t * P, (t + 1) * P)
                qsl = slice(sblk0 + f0, sblk0 + 512)
                sacc = ps2.tile([P, 2, 512], F32, tag="s2", name="sacc", bufs=2)
                nc.tensor.matmul(sacc[:, 0, f0:], kn[0][:, tsl], qn[0][:, qsl],
                                 start=True, stop=False)
                nc.tensor.matmul(sacc[:, 1, f0:], kn[1][:, tsl], qn[1][:, qsl],
                                 start=True, stop=False)
                nc.tensor.matmul(sacc[:, 0, f0:], kpe2[:ROPE, tsl], qpb2[:ROPE, qsl],
                                 start=False, stop=True)
                nc.tensor.matmul(sacc[:, 1, f0:], kpe2[ROPE:, tsl], qpb2[ROPE:, qsl],
                                 start=False, stop=True)
                if j >= 0:
                    nc.vector.tensor_add(sacc[:], sacc[:], masks[:, j])
                ex = epool.tile([P, 2, 512], F16, name="ex")
                nc.scalar.activation(ex[:], sacc[:], Exp, scale=SCALE, bias=negc[:])
                exq.append(ex)

            def emit_consume(b, t, zac, oac0, oac1):
                n_tc = 4 * (b + 1)
                ex = exq.pop(0)
                st, sp = (t == 0), (t == n_tc - 1)
                nc.tensor.matmul(zac[0:1, :], onesc_f[:], ex[:, 0, :], start=st, stop=sp)
                nc.tensor.matmul(zac[32:33, :], onesc_f[:], ex[:, 1, :], start=st, stop=sp)
                nc.tensor.matmul(oac0[:], vv[:, t, :VDIM], ex[:, 0, :], start=st, stop=sp)
                nc.tensor.matmul(oac1[:], vv[:, t, VDIM:], ex[:, 1, :], start=st, stop=sp)

            ci = 0
            for b in range(N_SB):
                sblk = slice(b * 512, (b + 1) * 512)
                n_tc = 4 * (b + 1)
                zac = ps2.tile([ROPE, 512], F32, tag="zacc", name="zac")
                oac0 = ps2.tile([P, 512], F32, tag="oacc0", name="oac0")
                oac1 = ps2.tile([P, 512], F32, tag="oacc1", name="oac1")
                if b == 0:
                    emit_scores(0, 0)
                    if n_tc > 1:
                        emit_scores(0, 1)
                for t in range(n_tc):
                    # global 2-deep lookahead across block boundaries
                    la = ci + 2
                    if la < len(chunks):
                        emit_scores(*chunks[la])
                    emit_consume(b, t, zac, oac0, oac1)
                    ci += 1
                # block epilogue: deferred softmax normalization.
                # one reciprocal covers both heads' z rows (0 and 32);
                # per-head bcast matmuls use base-partition-matched ones rows.
                rz2 = work.tile([33, 512], F32R, name="rz2")
                with nc.allow_low_precision(reason="float32r is bitwise float32"):
                    nc.vector.reciprocal(rz2[:], zac[0:33, :])
                for h in range(2):
                    rsl = slice(0, 1) if h == 0 else slice(32, 33)
                    bcz = ps2.tile([P, 512], F32, tag="bcast", name="bcz")
                    nc.tensor.matmul(bcz[:], onesr33[rsl, :], rz2[rsl, :],
                                     start=True, stop=True)
                    bczs = work.tile([P, 512], F32, name="bczs")
                    nc.scalar.activation(bczs[:], bcz[:],
                                         mybir.ActivationFunctionType.Copy)
                    oac = oac0 if h == 0 else oac1
                    with nc.allow_low_precision(reason="fp16 attention output"):
                        nc.vector.tensor_mul(ao[h][:, sblk], oac[:], bczs[:])

            # ---------------- o_proj partial ----------------
            for nb in range(N_SB):
                osl = slice(nb * 512, (nb + 1) * 512)
                for sc in range(N_SC):
                    ssl = slice(sc * P, (sc + 1) * P)
                    acc = ps2.tile([P, 512], F32, tag="s2", name="oo_acc", bufs=2)
                    nc.tensor.matmul(acc[:], ao[0][:, ssl], wos[:, 0, osl],
                                     start=True, stop=False)
                    nc.tensor.matmul(acc[:], ao[1][:, ssl], wos[:, 1, osl],
                                     start=False, stop=True)
                    ot = work.tile([P, 512], F16, name="ot", bufs=4)
                    if sc % 2 == 0:
                        with nc.allow_low_precision(reason="fp16 output partial"):
                            nc.vector.tensor_copy(ot[:], acc[:])
                    else:
                        # ACT-side eviction keeps DVE off the o_proj critical path
                        nc.scalar.activation(ot[:], acc[:],
                                             mybir.ActivationFunctionType.Copy)
                    nc.sync.dma_start(yout[ssl, osl], ot[:])


_CACHED = None


def _get_program():
    global _CACHED
    if _CACHED is None:
        _CACHED = build_program()
    return _CACHED


def _host_prep(hidden_states, w_qkv_a, q_a_ln_w, w_q_b, w_kv_b, kv_a_ln_w, w_o,
               positions):
    f32 = np.float32
    f16 = np.float16
    hs = np.asarray(hidden_states, dtype=f32)
    w1m = np.asarray(w_qkv_a, dtype=f32)
    wqm = np.asarray(w_q_b, dtype=f32) * np.asarray(q_a_ln_w, f32)[None, :]
    wkvm = np.asarray(w_kv_b, dtype=f32) * np.asarray(kv_a_ln_w, f32)[None, :]
    wom = np.asarray(w_o, dtype=f32)

    # rope tables (interleaved / non-neox), matching the reference fp32 math
    pos = np.asarray(positions).astype(f32)
    inv_freq = (1.0 / (f32(THETA) ** (np.arange(0, ROPE, 2, dtype=f32) / f32(ROPE)))).astype(f32)
    fr = pos[None, :] * inv_freq[:, None]              # [32, S]
    cos = np.cos(fr).astype(f32)
    sin = np.sin(fr).astype(f32)
    cosT = np.repeat(cos, 2, axis=0)                   # [64, S]
    ssinT = np.empty((ROPE, SEQ), f32)
    ssinT[0::2] = -sin
    ssinT[1::2] = sin
    psw = np.zeros((ROPE, ROPE), f32)                  # lhsT: out = psw.T @ x
    for i in range(0, ROPE, 2):
        psw[i + 1, i] = 1.0                            # out[i]   = x[i+1]
        psw[i, i + 1] = 1.0                            # out[i+1] = x[i]
    psw2 = np.zeros((P, P), f32)
    psw2[:ROPE, :ROPE] = psw
    psw2[ROPE:, ROPE:] = psw
    csq2 = np.concatenate([cosT, cosT], axis=0)        # [128, S] both heads
    ssq2 = np.concatenate([ssinT, ssinT], axis=0)

    hT = hs.T                                          # [I, S]
    # pad w_qkv_a^T out-dim 2112 -> 2176 (17*128); cols past 2112 are zero.
    w1T = np.zeros((HIDDEN, 17 * P), f32)
    w1T[:, :QLR + KVLR + ROPE] = w1m.T
    w1l = np.ascontiguousarray(
        w1T.reshape(N_KC, P, 17, P).transpose(2, 1, 0, 3).reshape(17, P, HIDDEN)).astype(f16)
    wq4 = wqm.reshape(H, QK, QLR)
    wkv4 = wkvm.reshape(H, NOPE + VDIM, KVLR)

    in_maps = []
    for c in range(N_CORES):
        ssl = slice(c * SSH, (c + 1) * SSH)
        h1 = np.ascontiguousarray(hT[:, ssl].reshape(N_KC, P, SSH).transpose(1, 0, 2)).astype(f16)
        # q B-proj columns per k-chunk: [nope_h0 | nope_h1 | rope_h0 | rope_h1]
        wh0, wh1 = wq4[HPC * c], wq4[HPC * c + 1]      # [QK, QLR]
        wq_cols = np.concatenate([wh0[:NOPE], wh1[:NOPE],
                                  wh0[NOPE:], wh1[NOPE:]], axis=0)  # [384, QLR]
        wql = np.ascontiguousarray(
            wq_cols.T.reshape(N_QAC, P, HPC * QK).transpose(1, 0, 2)).astype(f16)
        # kv columns per k-chunk: [kn_h0 | kn_h1 | v_h0 | v_h1]
        wkvc = wkv4[HPC * c:HPC * (c + 1)]             # [2, 256, 512]
        wkv_cols = np.concatenate([wkvc[0, :NOPE], wkvc[1, :NOPE],
                                   wkvc[0, NOPE:], wkvc[1, NOPE:]], axis=0)
        wkvl = np.ascontiguousarray(
            wkv_cols.T.reshape(N_KVC, P, HPC * (NOPE + VDIM)).transpose(1, 0, 2)).astype(f16)
        woc = wom[:, HPC * VDIM * c:HPC * VDIM * (c + 1)].T          # [256, 2048]
        wol = np.ascontiguousarray(woc.reshape(HPC, P, HIDDEN).transpose(1, 0, 2)).astype(f16)
        in_maps.append({
            "h1": h1, "w1": w1l, "wq": wql, "wkv": wkvl, "wo": wol,
            "csq2": csq2.astype(f16), "ssq2": ssq2.astype(f16),
            "clc": np.ascontiguousarray(cosT[:, ssl]).astype(f16),
            "cls": np.ascontiguousarray(ssinT[:, ssl]).astype(f16),
            "psw1": psw.astype(f16), "psw2": psw2.astype(f16),
            "onesc": np.ones((P, 1), f16),
            "onesr": np.ones((1, P), f16),
            "onesrr": np.ones((33, P), f32),
        })
    return in_maps


def kernel(**inputs):
    nc = _get_program()
    in_maps = _host_prep(**inputs)
    res = run_bass_kernel_spmd(nc, in_maps, list(range(N_CORES)))
    out = np.zeros((SEQ, HIDDEN), np.float64)
    for c in range(N_CORES):
        out += res.results[c]["y"].astype(np.float64)
    return out.astype(np.float32)
